# revision 7
# baseline (speedup 1.0000x reference)
"""Trainium2 Bass kernel for ARboxMultiRoIExtractor (RoIAlignRotated over 4 FPN
levels + cross-level max).

Strategy: data-parallel over rois (64 per core x 8 cores), FPN features
replicated in NHWC layout.  For each (roi-half chunk, level) the device does a
dma_gather of 8 taps x 128 bins (each tap = 2 adjacent pixels x 256 channels),
a weighted reduce on ACT/DVE, a cross-level max, a PE transpose to [C, bins]
and a DMA to the output.  All indices/weights are computed host-side from rois
and shipped as per-core input tensors; the per-chunk DRAM gather base is read
from a per-core tensor at runtime into a GPSIMD register (keeps the program
SPMD-uniform while staying within int16 gather indices).
"""
import sys

sys.path.insert(0, "/opt/trn_rl_repo")

from contextlib import ExitStack

import numpy as np

import concourse.bass as bass
import concourse.tile as tile
from concourse import bacc, mybir
from concourse._compat import cdiv
from concourse.bass_utils import run_bass_kernel_spmd
from concourse.library_config import mlp
from concourse.masks import make_identity

# ---- problem constants (hardcoded; kernel.py must be self-contained) ----
B = 2
C = 256
N_ROIS = 512
N_CORES = 8
OUT_H, OUT_W, NS = 7, 35, 2
N_BINS = OUT_H * OUT_W  # 245
STRIDES = (4, 8, 16, 32)
HW_L = ((256, 256), (128, 128), (64, 64), (32, 32))
N_LEVELS = 4
N_SLOTS = 8               # 4 subsamples x 2 rows
ELEM = 2 * C              # gathered element: 2 adjacent pixels, all channels
IDX_PER_CHUNK = N_SLOTS * 128          # 1024
IDX_COLS = IDX_PER_CHUNK // 16         # 64
WTS_PER_CHUNK = N_SLOTS * 2            # 16
J_SPLIT = 18                           # chunk = (roi, j<18 | j>=18)
# gather-window row counts per level (int16 index space)
N_ROWS_L = tuple(min(32768, B * H * W) for H, W in HW_L)


# ------------------------------------------------------------------
# host-side index/weight computation
# ------------------------------------------------------------------
def _prep_level(rois, level):
    """rois: [R, 6] float32.  Returns (idx, wts, base) for one level:
    idx  [128, 2R*IDX_COLS] int16  (gather indices, dma_gather layout)
    wts  [128, 2R*WTS_PER_CHUNK] float32
    base [1, 2R] int32 (per-chunk dram row base)
    """
    R = rois.shape[0]
    H, W = HW_L[level]
    NPX = B * H * W
    scale = np.float32(1.0 / STRIDES[level])

    b = rois[:, 0].astype(np.int32)
    cx = rois[:, 1] * scale
    cy = rois[:, 2] * scale
    w = np.maximum(rois[:, 3] * scale, np.float32(1.0))
    h = np.maximum(rois[:, 4] * scale, np.float32(1.0))
    th = rois[:, 5]
    bin_h = h / np.float32(OUT_H)
    bin_w = w / np.float32(OUT_W)

    ar = lambda n: np.arange(n, dtype=np.float32)
    gy = ar(OUT_H)[:, None, None, None] + (ar(NS)[None, None, :, None] + np.float32(0.5)) / np.float32(NS)
    gx = ar(OUT_W)[None, :, None, None] + (ar(NS)[None, None, None, :] + np.float32(0.5)) / np.float32(NS)
    r = lambda v: v[:, None, None, None, None]
    yy = -r(h) * np.float32(0.5) + gy[None] * r(bin_h)          # [R,7,1,2,1]
    xx = -r(w) * np.float32(0.5) + gx[None] * r(bin_w)          # [R,1,35,1,2]
    cos_t, sin_t = r(np.cos(th)), r(np.sin(th))
    x = (xx * cos_t - yy * sin_t + r(cx)).astype(np.float32)    # [R,7,35,2,2]
    y = (xx * sin_t + yy * cos_t + r(cy)).astype(np.float32)
    x = np.broadcast_to(x, (R, OUT_H, OUT_W, NS, NS))
    y = np.broadcast_to(y, (R, OUT_H, OUT_W, NS, NS))

    valid = (y >= -1.0) & (y <= H) & (x >= -1.0) & (x <= W)
    yc = np.clip(y, np.float32(0.0), np.float32(H - 1))
    xc = np.clip(x, np.float32(0.0), np.float32(W - 1))
    y0 = np.floor(yc).astype(np.int32)
    x0 = np.floor(xc).astype(np.int32)
    y1 = np.minimum(y0 + 1, H - 1)
    ly = yc - y0
    lx = xc - x0
    hy = np.float32(1.0) - ly
    hx = np.float32(1.0) - lx

    q = np.float32(0.25) * valid.astype(np.float32)
    # weights per (row r, lr): r=0 -> hy, r=1 -> ly; lr=0 -> hx, lr=1 -> lx
    wl0 = hy * hx * q
    wr0 = hy * lx * q
    wl1 = ly * hx * q
    wr1 = ly * lx * q

    bb = b[:, None, None, None, None]
    px0 = (bb * H + y0) * W + x0          # row y0 gather px
    px1 = (bb * H + y1) * W + x0          # row y1 gather px

    # -> [R, 245, 4 subs]
    def fl(a):
        return a.reshape(R, N_BINS, NS * NS)

    px = np.stack([fl(px0), fl(px1)], axis=3).reshape(R, N_BINS, N_SLOTS)      # slot = sub*2+r
    wl = np.stack([fl(wl0), fl(wl1)], axis=3).reshape(R, N_BINS, N_SLOTS)
    wr = np.stack([fl(wr0), fl(wr1)], axis=3).reshape(R, N_BINS, N_SLOTS)

    # bins within a chunk: j-half split (keeps the chunk's image-row span
    # small for any rotation angle), i-major within the half
    px_g = px.reshape(R, OUT_H, OUT_W, N_SLOTS)
    wl_g = wl.reshape(R, OUT_H, OUT_W, N_SLOTS)
    wr_g = wr.reshape(R, OUT_H, OUT_W, N_SLOTS)

    n_chunks = 2 * R
    idx = np.zeros((128, n_chunks * IDX_COLS), np.int16)
    wts = np.zeros((128, n_chunks * WTS_PER_CHUNK), np.float32)
    bases = np.zeros((1, n_chunks), np.int32)

    max_base = NPX - N_ROWS_L[level]
    for roi in range(R):
        for half in range(2):
            ci = roi * 2 + half
            j0, j1 = (0, J_SPLIT) if half == 0 else (J_SPLIT, OUT_W)
            nb = OUT_H * (j1 - j0)
            pxc = px_g[roi, :, j0:j1].reshape(nb, N_SLOTS)
            base = min(int(pxc.min()) // W * W, max_base)
            base = max(base, 0)
            bases[0, ci] = base
            rel = pxc - base
            assert rel.min() >= 0 and rel.max() < 32768, (level, roi, half, rel.min(), rel.max())
            # stream position i = slot*128 + p  ->  idx[i%16, ci*IDX_COLS + i//16]
            ii = np.arange(N_SLOTS)[None, :] * 128 + np.arange(nb)[:, None]  # [nb, 8]
            cols = ci * IDX_COLS + ii // 16
            rows = ii % 16
            idx[rows, cols] = rel.astype(np.int16)
            wcol = ci * WTS_PER_CHUNK + np.arange(N_SLOTS) * 2
            wts[:nb, wcol] = wl_g[roi, :, j0:j1].reshape(nb, N_SLOTS)
            wts[:nb, wcol + 1] = wr_g[roi, :, j0:j1].reshape(nb, N_SLOTS)
    idx[16:] = np.tile(idx[:16], (7, 1))
    return idx, wts, bases


def prep_core_inputs(rois_shard):
    out = {}
    for l in range(N_LEVELS):
        idx, wts, base = _prep_level(rois_shard, l)
        out[f"idx{l}"] = idx
        out[f"wts{l}"] = wts
        out[f"base{l}"] = base
    return out


# ------------------------------------------------------------------
# device program
# ------------------------------------------------------------------
def build_program(R):
    """R = rois per core.  SPMD-uniform program."""
    n_chunks = 2 * R
    nc = bacc.Bacc("TRN2", debug=False)

    feats = []
    for l in range(N_LEVELS):
        H, W = HW_L[l]
        feats.append(nc.dram_tensor(f"f{l}", [B * H * W + W, C], mybir.dt.float32, kind="ExternalInput"))
    idx_d, wts_d, base_d = [], [], []
    for l in range(N_LEVELS):
        idx_d.append(nc.dram_tensor(f"idx{l}", [128, n_chunks * IDX_COLS], mybir.dt.int16, kind="ExternalInput"))
        wts_d.append(nc.dram_tensor(f"wts{l}", [128, n_chunks * WTS_PER_CHUNK], mybir.dt.float32, kind="ExternalInput"))
        base_d.append(nc.dram_tensor(f"base{l}", [1, n_chunks], mybir.dt.int32, kind="ExternalInput"))
    out_d = nc.dram_tensor("out", [R, C, N_BINS], mybir.dt.float32, kind="ExternalOutput")

    with tile.TileContext(nc) as tc, ExitStack() as ctx:
        nc.gpsimd.load_library(mlp)

        const_pool = ctx.enter_context(tc.tile_pool(name="const", bufs=1))
        meta_pool = ctx.enter_context(tc.tile_pool(name="meta", bufs=1))
        dst_pool = ctx.enter_context(tc.tile_pool(name="dst", bufs=3))
        acc_pool = ctx.enter_context(tc.tile_pool(name="acc", bufs=2))
        tmp_pool = ctx.enter_context(tc.tile_pool(name="tmp", bufs=4))
        res_pool = ctx.enter_context(tc.tile_pool(name="res", bufs=2))
        psum_pool = ctx.enter_context(tc.tile_pool(name="psum", bufs=2, space="PSUM"))
        outp = ctx.enter_context(tc.tile_pool(name="outp", bufs=3))

        identity = const_pool.tile([128, 128], mybir.dt.float32)
        make_identity(nc, identity[:])

        idx_sb, wts_sb, base_sb = [], [], []
        for l in range(N_LEVELS):
            t = meta_pool.tile([128, n_chunks * IDX_COLS], mybir.dt.int16, tag=f"idx{l}")
            nc.sync.dma_start(t[:], idx_d[l][:])
            idx_sb.append(t)
            t = meta_pool.tile([128, n_chunks * WTS_PER_CHUNK], mybir.dt.float32, tag=f"wts{l}")
            nc.sync.dma_start(t[:], wts_d[l][:])
            wts_sb.append(t)
            t = meta_pool.tile([1, n_chunks], mybir.dt.int32, tag=f"base{l}")
            nc.sync.dma_start(t[:], base_d[l][:])
            base_sb.append(t)

        breg = nc.gpsimd.alloc_register("base_reg")

        for ci in range(n_chunks):
            roi, half = divmod(ci, 2)
            j0 = 0 if half == 0 else J_SPLIT
            nj = (J_SPLIT - j0) if half == 0 else (OUT_W - J_SPLIT)
            nb = OUT_H * nj
            res = res_pool.tile([128, C], mybir.dt.float32, tag="res")
            for l in range(N_LEVELS):
                nc.gpsimd.reg_load(breg, base_sb[l][0:1, ci:ci + 1])
                bval = nc.gpsimd.snap(breg)
                src_ap = bass.AP(feats[l], bval * C, [[C, N_ROWS_L[l]], [1, ELEM]])
                dst = dst_pool.tile([128, N_SLOTS, ELEM], mybir.dt.float32, tag="dst")
                nc.gpsimd.dma_gather(
                    dst[:], src_ap, idx_sb[l][:, ci * IDX_COLS:(ci + 1) * IDX_COLS],
                    IDX_PER_CHUNK, IDX_PER_CHUNK, ELEM, elem_step=C,
                )
                wts_c = wts_sb[l]
                wbase = ci * WTS_PER_CHUNK
                acc = acc_pool.tile([128, C], mybir.dt.float32, tag="acc")
                # k-th term: dst[:, k//2, (k%2)*C:...] * wts[:, wbase+k]
                nc.scalar.activation(
                    acc[:], dst[:, 0, 0:C], mybir.ActivationFunctionType.Copy,
                    scale=wts_c[:, wbase:wbase + 1],
                )
                for k in range(1, 2 * N_SLOTS):
                    t = tmp_pool.tile([128, C], mybir.dt.float32, tag="tmp")
                    g = dst[:, k // 2, (k % 2) * C:(k % 2) * C + C]
                    wap = wts_c[:, wbase + k:wbase + k + 1]
                    if k % 4 == 3:   # offload some scales to DVE to balance engines
                        nc.vector.tensor_scalar(t[:], g, wap, None, mybir.AluOpType.mult)
                    else:
                        nc.scalar.activation(t[:], g, mybir.ActivationFunctionType.Copy, scale=wap)
                    nc.vector.tensor_add(acc[:], acc[:], t[:])
                if l == 0:
                    nc.vector.tensor_copy(res[:], acc[:])
                else:
                    nc.vector.tensor_tensor(res[:], res[:], acc[:], op=mybir.AluOpType.max)
            for ch in range(2):
                ps = psum_pool.tile([128, 128], mybir.dt.float32, tag="ps")
                nc.tensor.transpose(ps[:], res[:, ch * 128:(ch + 1) * 128], identity[:])
                ot = outp.tile([128, 128], mybir.dt.float32, tag="ot")
                nc.scalar.copy(ot[:], ps[:])
                # out[roi, ch*128 + c, i*35 + j0 + jj] = ot[c, i*nj + jj]
                dst_ap = bass.AP(
                    out_d,
                    roi * C * N_BINS + (ch * 128) * N_BINS + j0,
                    [[N_BINS, 128], [OUT_W, OUT_H], [1, nj]],
                )
                nc.sync.dma_start(dst_ap, ot[:, 0:nb].rearrange("p (i j) -> p i j", i=OUT_H))
    nc.compile()
    return nc


_PROGRAM_CACHE = {}


def _get_program(R):
    if R not in _PROGRAM_CACHE:
        _PROGRAM_CACHE[R] = build_program(R)
    return _PROGRAM_CACHE[R]


def kernel(feat0, feat1, feat2, feat3, rois):
    feats = [np.asarray(f, np.float32) for f in (feat0, feat1, feat2, feat3)]
    rois = np.asarray(rois, np.float32)
    R = N_ROIS // N_CORES

    feat_flat = []
    for l, f in enumerate(feats):
        H, W = HW_L[l]
        nhwc = np.ascontiguousarray(f.transpose(0, 2, 3, 1)).reshape(B * H * W, C)
        nhwc = np.concatenate([nhwc, np.zeros((W, C), np.float32)], axis=0)
        feat_flat.append(nhwc)

    nc = _get_program(R)

    in_maps = []
    for k in range(N_CORES):
        m = {f"f{l}": feat_flat[l] for l in range(N_LEVELS)}
        m.update(prep_core_inputs(rois[k * R:(k + 1) * R]))
        in_maps.append(m)

    import os
    trace = bool(int(os.environ.get("KERNEL_TRACE", "0")))
    res = run_bass_kernel_spmd(nc, in_maps, list(range(N_CORES)), trace=trace)
    global LAST_RESULT, LAST_NC, LAST_IN_MAPS
    LAST_RESULT = res
    LAST_NC = nc
    LAST_IN_MAPS = in_maps
    outs = [res.results[k]["out"] for k in range(N_CORES)]
    full = np.concatenate(outs, axis=0)                      # [512, 256, 245]
    return np.ascontiguousarray(full.reshape(N_ROIS, C, OUT_H, OUT_W))
